# revision 2
# baseline (speedup 1.0000x reference)
"""MoE (dense-routing reference) Trainium2 kernel, expert-parallel across 8 cores.

Strategy (per sharding hint: token dispatch by top-k expert id):
  - Host (numpy): router logits -> top-2 experts + probs per token, aux loss.
  - Dispatch: gather each expert's routed tokens into a padded batch.
  - Device (8 NeuronCores, SPMD, expert e on core e): SwiGLU FFN
      y = (silu(x @ w1 + b1) * (x @ w2 + b2)) @ w3
    in bf16 matmuls with fp32 PSUM accumulation.
  - Host: combine  out[tok] += prob * (y + b3)  and return (output, aux_loss).

The reference runs every expert densely on all 8192 tokens; only the top-2
experts per token contribute to the output, so routed dispatch does ~1/4 the
FLOPs with identical math on the contributing terms.
"""

import numpy as np
import ml_dtypes

import concourse.bass as bass  # noqa: F401  (bass types via bacc/tile)
import concourse.mybir as mybir
import concourse.tile as tile
from concourse import bacc
from concourse.bass_utils import run_bass_kernel_spmd

BF16 = ml_dtypes.bfloat16
F32 = mybir.dt.float32
BF = mybir.dt.bfloat16

D_MODEL, D_HIDDEN, N_EXPERTS, TOP_K = 1024, 2048, 8, 2
B, T = 4, 2048
P = 128
KD = D_MODEL // P   # 8  k-tiles over d_model (contraction for w1/w2)
MH = D_HIDDEN // P  # 16 m-tiles over d_hidden
MD = D_MODEL // P   # 8  m-tiles over d_model (output of w3)

# Set by test harness to capture profile/exec time.
TRACE = False
LAST_RESULT = None

_NC_CACHE = {}


def _token_tiles(C):
    """Cover [0, C) with 512-wide tiles plus one >=256 tail (C % 256 == 0)."""
    tiles = []
    off = 0
    while off < C:
        w = min(512, C - off)
        tiles.append((off, w))
        off += w
    return tiles


def build_nc(C, debug=False):
    """Build the per-core Bass program for capacity-C token batches."""
    ttiles = _token_tiles(C)
    nc = bacc.Bacc(None, target_bir_lowering=False, debug=debug)

    xT = nc.dram_tensor("xT", [KD, P, C], BF, kind="ExternalInput")
    w1s = nc.dram_tensor("w1s", [MH, P, KD * P], BF, kind="ExternalInput")
    w2s = nc.dram_tensor("w2s", [MH, P, KD * P], BF, kind="ExternalInput")
    w3s = nc.dram_tensor("w3s", [MD, P, MH * P], BF, kind="ExternalInput")
    b1s = nc.dram_tensor("b1s", [P, MH], F32, kind="ExternalInput")
    b2s = nc.dram_tensor("b2s", [P, MH], F32, kind="ExternalInput")
    yT = nc.dram_tensor("yT", [MD, P, C], F32, kind="ExternalOutput")

    with tile.TileContext(nc) as tc:
        with (
            tc.tile_pool(name="xp", bufs=1) as xp,
            tc.tile_pool(name="hp", bufs=1) as hp,
            tc.tile_pool(name="cp", bufs=1) as cp,
            tc.tile_pool(name="wp", bufs=3) as wp,
            tc.tile_pool(name="w3p", bufs=3) as w3p,
            tc.tile_pool(name="sp", bufs=4) as sp,
            tc.tile_pool(name="yp", bufs=4) as yp,
            tc.tile_pool(name="ps", bufs=2, space="PSUM") as ps,
            tc.tile_pool(name="psy", bufs=3, space="PSUM") as psy,
        ):
            # Activations resident in SBUF: x^T (bf16) and h^T (bf16).
            xts = xp.tile([P, KD, C], BF)
            for k in range(KD):
                nc.sync.dma_start(xts[:, k, :], xT[k])
            b1t = cp.tile([P, MH], F32)
            nc.sync.dma_start(b1t[:], b1s[:])
            b2t = cp.tile([P, MH], F32)
            nc.sync.dma_start(b2t[:], b2s[:])
            hts = hp.tile([P, MH, C], BF)

            # Phase A: h = silu(x@w1 + b1) * (x@w2 + b2), produced m-tile-major.
            for m in range(MH):
                w1t = wp.tile([P, KD * P], BF, tag="w1t")
                nc.sync.dma_start(w1t[:], w1s[m])
                w2t = wp.tile([P, KD * P], BF, tag="w2t")
                nc.sync.dma_start(w2t[:], w2s[m])
                for off, w in ttiles:
                    ps1 = ps.tile([P, 512], F32, tag="ps1")
                    ps2 = ps.tile([P, 512], F32, tag="ps2")
                    for k in range(KD):
                        nc.tensor.matmul(
                            ps1[:, :w],
                            w1t[:, k * P : (k + 1) * P],
                            xts[:, k, off : off + w],
                            start=(k == 0),
                            stop=(k == KD - 1),
                        )
                    for k in range(KD):
                        nc.tensor.matmul(
                            ps2[:, :w],
                            w2t[:, k * P : (k + 1) * P],
                            xts[:, k, off : off + w],
                            start=(k == 0),
                            stop=(k == KD - 1),
                        )
                    # silu(v) = v * sigmoid(v) with v = ps1 + b1
                    s1 = sp.tile([P, 512], F32, tag="s1")
                    nc.scalar.activation(
                        s1[:, :w],
                        ps1[:, :w],
                        mybir.ActivationFunctionType.Sigmoid,
                        bias=b1t[:, m : m + 1],
                    )
                    t1 = sp.tile([P, 512], F32, tag="t1")
                    nc.vector.scalar_tensor_tensor(
                        t1[:, :w],
                        ps1[:, :w],
                        b1t[:, m : m + 1],
                        s1[:, :w],
                        mybir.AluOpType.add,
                        mybir.AluOpType.mult,
                    )
                    # h = (ps2 + b2) * silu(ps1 + b1), cast to bf16 on write.
                    nc.vector.scalar_tensor_tensor(
                        hts[:, m, off : off + w],
                        ps2[:, :w],
                        b2t[:, m : m + 1],
                        t1[:, :w],
                        mybir.AluOpType.add,
                        mybir.AluOpType.mult,
                    )

            # Phase B: y = h @ w3 (b3 + prob weighting applied on host).
            for dm in range(MD):
                w3t = w3p.tile([P, MH * P], BF, tag="w3t")
                nc.sync.dma_start(w3t[:], w3s[dm])
                for off, w in ttiles:
                    py = psy.tile([P, 512], F32, tag="py")
                    for kh in range(MH):
                        nc.tensor.matmul(
                            py[:, :w],
                            w3t[:, kh * P : (kh + 1) * P],
                            hts[:, kh, off : off + w],
                            start=(kh == 0),
                            stop=(kh == MH - 1),
                        )
                    yt = yp.tile([P, 512], F32, tag="yt")
                    nc.vector.tensor_copy(yt[:, :w], py[:, :w])
                    nc.sync.dma_start(yT[dm][:, off : off + w], yt[:, :w])

    nc.compile()
    return nc


def _route(x_flat, gate_w, gate_b):
    """numpy router: top-2 experts + renormalized probs per token, aux loss."""
    logits = x_flat @ gate_w + gate_b  # [N, E] fp32
    order = np.argsort(-logits, axis=-1, kind="stable")
    idx2 = order[:, :TOP_K]  # [N, 2] descending logit
    l2 = np.take_along_axis(logits, idx2, axis=1)
    e2 = np.exp(l2 - l2[:, :1])  # stable: subtract max (col 0)
    p2 = e2 / e2.sum(axis=1, keepdims=True)

    # Load-balance aux loss (exactly the reference formula).
    lmax = logits.max(axis=1, keepdims=True)
    el = np.exp(logits - lmax)
    gate_probs = el / el.sum(axis=1, keepdims=True)
    importance = gate_probs.mean(axis=0)
    load = np.bincount(idx2.reshape(-1), minlength=N_EXPERTS).astype(np.float32)
    load /= idx2.size
    aux_loss = np.float32((importance * load).sum() * N_EXPERTS)
    return idx2, p2.astype(np.float32), aux_loss


def kernel(x, gate_w, gate_b, w1, b1, w2, b2, w3, b3):
    global LAST_RESULT
    x = np.asarray(x, np.float32)
    x_flat = x.reshape(-1, D_MODEL)
    N = x_flat.shape[0]

    idx2, p2, aux_loss = _route(x_flat, np.asarray(gate_w, np.float32), np.asarray(gate_b, np.float32))

    # Per-expert token lists + coefficients.
    toks, coefs = [], []
    for e in range(N_EXPERTS):
        r0 = np.nonzero(idx2[:, 0] == e)[0]
        r1 = np.nonzero(idx2[:, 1] == e)[0]
        toks.append(np.concatenate([r0, r1]))
        coefs.append(np.concatenate([p2[r0, 0], p2[r1, 1]]).astype(np.float32))

    max_load = max(len(t) for t in toks)
    C = max(512, -(-max_load // 256) * 256)  # pad to multiple of 256, >= 512

    if C not in _NC_CACHE:
        _NC_CACHE[C] = build_nc(C)
    nc = _NC_CACHE[C]

    w1 = np.asarray(w1, np.float32)
    w2 = np.asarray(w2, np.float32)
    w3 = np.asarray(w3, np.float32)
    b1 = np.asarray(b1, np.float32)
    b2 = np.asarray(b2, np.float32)
    b3 = np.asarray(b3, np.float32)

    in_maps = []
    for e in range(N_EXPERTS):
        n_e = len(toks[e])
        xg = np.zeros((C, D_MODEL), np.float32)
        xg[:n_e] = x_flat[toks[e]]
        xT = np.ascontiguousarray(xg.T).reshape(KD, P, C).astype(BF16)
        # w1s[m, p, k*P+j] = w1[k*P+p, m*P+j]
        w1s = np.ascontiguousarray(
            w1[e].reshape(KD, P, MH, P).transpose(2, 1, 0, 3).reshape(MH, P, KD * P)
        ).astype(BF16)
        w2s = np.ascontiguousarray(
            w2[e].reshape(KD, P, MH, P).transpose(2, 1, 0, 3).reshape(MH, P, KD * P)
        ).astype(BF16)
        # w3s[dm, p, kh*P+j] = w3[kh*P+p, dm*P+j]
        w3s = np.ascontiguousarray(
            w3[e].reshape(MH, P, MD, P).transpose(2, 1, 0, 3).reshape(MD, P, MH * P)
        ).astype(BF16)
        b1s = np.ascontiguousarray(b1[e].reshape(MH, P).T)
        b2s = np.ascontiguousarray(b2[e].reshape(MH, P).T)
        in_maps.append(
            {"xT": xT, "w1s": w1s, "w2s": w2s, "w3s": w3s, "b1s": b1s, "b2s": b2s}
        )

    res = run_bass_kernel_spmd(nc, in_maps, core_ids=list(range(N_EXPERTS)), trace=TRACE)
    LAST_RESULT = res

    out_flat = np.zeros((N, D_MODEL), np.float32)
    for e in range(N_EXPERTS):
        n_e = len(toks[e])
        yT = np.asarray(res.results[e]["yT"], np.float32).reshape(D_MODEL, C)
        y = yT.T[:n_e]  # [n_e, D]
        out_flat[toks[e]] += coefs[e][:, None] * (y + b3[e][None, :])

    return out_flat.reshape(x.shape), aux_loss


# revision 3
# speedup vs baseline: 1.0014x; 1.0014x over previous
"""MoE (dense-routing reference) Trainium2 kernel, expert-parallel across 8 cores.

Strategy (per sharding hint: token dispatch by top-k expert id):
  - Host (numpy): router logits -> top-2 experts + probs per token, aux loss.
  - Dispatch: gather each expert's routed tokens into a padded batch.
  - Device (8 NeuronCores, SPMD, expert e on core e): SwiGLU FFN
      y = (silu(x @ w1 + b1) * (x @ w2 + b2)) @ w3
    in bf16 matmuls with fp32 PSUM accumulation.
  - Host: combine  out[tok] += prob * (y + b3)  and return (output, aux_loss).

The reference runs every expert densely on all 8192 tokens; only the top-2
experts per token contribute to the output, so routed dispatch does ~1/4 the
FLOPs with identical math on the contributing terms.
"""

import numpy as np
import ml_dtypes

import concourse.bass as bass  # noqa: F401  (bass types via bacc/tile)
import concourse.mybir as mybir
import concourse.tile as tile
from concourse import bacc
from concourse.bass_utils import run_bass_kernel_spmd

BF16 = ml_dtypes.bfloat16
F32 = mybir.dt.float32
BF = mybir.dt.bfloat16

D_MODEL, D_HIDDEN, N_EXPERTS, TOP_K = 1024, 2048, 8, 2
B, T = 4, 2048
P = 128
KD = D_MODEL // P   # 8  k-tiles over d_model (contraction for w1/w2)
MH = D_HIDDEN // P  # 16 m-tiles over d_hidden
MD = D_MODEL // P   # 8  m-tiles over d_model (output of w3)

# Set by test harness to capture profile/exec time.
TRACE = False
LAST_RESULT = None

_NC_CACHE = {}


def _token_tiles(C):
    """Cover [0, C) with 512-wide tiles plus one >=256 tail (C % 256 == 0)."""
    tiles = []
    off = 0
    while off < C:
        w = min(512, C - off)
        tiles.append((off, w))
        off += w
    return tiles


def build_nc(C, debug=False):
    """Build the per-core Bass program for capacity-C token batches."""
    ttiles = _token_tiles(C)
    nc = bacc.Bacc(None, target_bir_lowering=False, debug=debug)

    xT = nc.dram_tensor("xT", [KD, P, C], BF, kind="ExternalInput")
    w1s = nc.dram_tensor("w1s", [MH, P, KD * P], BF, kind="ExternalInput")
    w2s = nc.dram_tensor("w2s", [MH, P, KD * P], BF, kind="ExternalInput")
    w3s = nc.dram_tensor("w3s", [MD, P, MH * P], BF, kind="ExternalInput")
    b1s = nc.dram_tensor("b1s", [P, MH], F32, kind="ExternalInput")
    b2s = nc.dram_tensor("b2s", [P, MH], F32, kind="ExternalInput")
    yT = nc.dram_tensor("yT", [MD, P, C], F32, kind="ExternalOutput")

    with tile.TileContext(nc) as tc:
        with (
            tc.tile_pool(name="xp", bufs=1) as xp,
            tc.tile_pool(name="hp", bufs=1) as hp,
            tc.tile_pool(name="cp", bufs=1) as cp,
            tc.tile_pool(name="wp", bufs=3) as wp,
            tc.tile_pool(name="w3p", bufs=3) as w3p,
            tc.tile_pool(name="sp", bufs=4) as sp,
            tc.tile_pool(name="yp", bufs=4) as yp,
            tc.tile_pool(name="ps", bufs=2, space="PSUM") as ps,
            tc.tile_pool(name="psy", bufs=3, space="PSUM") as psy,
        ):
            # m=0 weights first so the first matmul isn't queued behind the
            # full x^T transfer.
            w1t0 = wp.tile([P, KD * P], BF, tag="w1t")
            nc.sync.dma_start(w1t0[:], w1s[0])
            w2t0 = wp.tile([P, KD * P], BF, tag="w2t")
            nc.sync.dma_start(w2t0[:], w2s[0])
            b1t = cp.tile([P, MH], F32)
            nc.sync.dma_start(b1t[:], b1s[:])
            b2t = cp.tile([P, MH], F32)
            nc.sync.dma_start(b2t[:], b2s[:])
            # Activations resident in SBUF: x^T (bf16) and h^T (bf16),
            # transferred in (token-tile, k) consumption order.
            xts = xp.tile([P, KD, C], BF)
            for off, w in ttiles:
                for k in range(KD):
                    nc.sync.dma_start(xts[:, k, off : off + w], xT[k][:, off : off + w])
            hts = hp.tile([P, MH, C], BF)

            # Phase A: h = silu(x@w1 + b1) * (x@w2 + b2), produced m-tile-major.
            for m in range(MH):
                if m == 0:
                    w1t, w2t = w1t0, w2t0
                else:
                    w1t = wp.tile([P, KD * P], BF, tag="w1t")
                    nc.sync.dma_start(w1t[:], w1s[m])
                    w2t = wp.tile([P, KD * P], BF, tag="w2t")
                    nc.sync.dma_start(w2t[:], w2s[m])
                for off, w in ttiles:
                    ps1 = ps.tile([P, 512], F32, tag="ps1")
                    ps2 = ps.tile([P, 512], F32, tag="ps2")
                    for k in range(KD):
                        nc.tensor.matmul(
                            ps1[:, :w],
                            w1t[:, k * P : (k + 1) * P],
                            xts[:, k, off : off + w],
                            start=(k == 0),
                            stop=(k == KD - 1),
                        )
                    for k in range(KD):
                        nc.tensor.matmul(
                            ps2[:, :w],
                            w2t[:, k * P : (k + 1) * P],
                            xts[:, k, off : off + w],
                            start=(k == 0),
                            stop=(k == KD - 1),
                        )
                    # silu(v) = v * sigmoid(v) with v = ps1 + b1
                    s1 = sp.tile([P, 512], F32, tag="s1")
                    nc.scalar.activation(
                        s1[:, :w],
                        ps1[:, :w],
                        mybir.ActivationFunctionType.Sigmoid,
                        bias=b1t[:, m : m + 1],
                    )
                    t1 = sp.tile([P, 512], F32, tag="t1")
                    nc.vector.scalar_tensor_tensor(
                        t1[:, :w],
                        ps1[:, :w],
                        b1t[:, m : m + 1],
                        s1[:, :w],
                        mybir.AluOpType.add,
                        mybir.AluOpType.mult,
                    )
                    # h = (ps2 + b2) * silu(ps1 + b1), cast to bf16 on write.
                    nc.vector.scalar_tensor_tensor(
                        hts[:, m, off : off + w],
                        ps2[:, :w],
                        b2t[:, m : m + 1],
                        t1[:, :w],
                        mybir.AluOpType.add,
                        mybir.AluOpType.mult,
                    )

            # Phase B: y = h @ w3 (b3 + prob weighting applied on host).
            for dm in range(MD):
                w3t = w3p.tile([P, MH * P], BF, tag="w3t")
                nc.sync.dma_start(w3t[:], w3s[dm])
                for off, w in ttiles:
                    py = psy.tile([P, 512], F32, tag="py")
                    for kh in range(MH):
                        nc.tensor.matmul(
                            py[:, :w],
                            w3t[:, kh * P : (kh + 1) * P],
                            hts[:, kh, off : off + w],
                            start=(kh == 0),
                            stop=(kh == MH - 1),
                        )
                    yt = yp.tile([P, 512], F32, tag="yt")
                    nc.vector.tensor_copy(yt[:, :w], py[:, :w])
                    nc.sync.dma_start(yT[dm][:, off : off + w], yt[:, :w])

    nc.compile()
    return nc


def _route(x_flat, gate_w, gate_b):
    """numpy router: top-2 experts + renormalized probs per token, aux loss."""
    logits = x_flat @ gate_w + gate_b  # [N, E] fp32
    order = np.argsort(-logits, axis=-1, kind="stable")
    idx2 = order[:, :TOP_K]  # [N, 2] descending logit
    l2 = np.take_along_axis(logits, idx2, axis=1)
    e2 = np.exp(l2 - l2[:, :1])  # stable: subtract max (col 0)
    p2 = e2 / e2.sum(axis=1, keepdims=True)

    # Load-balance aux loss (exactly the reference formula).
    lmax = logits.max(axis=1, keepdims=True)
    el = np.exp(logits - lmax)
    gate_probs = el / el.sum(axis=1, keepdims=True)
    importance = gate_probs.mean(axis=0)
    load = np.bincount(idx2.reshape(-1), minlength=N_EXPERTS).astype(np.float32)
    load /= idx2.size
    aux_loss = np.float32((importance * load).sum() * N_EXPERTS)
    return idx2, p2.astype(np.float32), aux_loss


def kernel(x, gate_w, gate_b, w1, b1, w2, b2, w3, b3):
    global LAST_RESULT
    x = np.asarray(x, np.float32)
    x_flat = x.reshape(-1, D_MODEL)
    N = x_flat.shape[0]

    idx2, p2, aux_loss = _route(x_flat, np.asarray(gate_w, np.float32), np.asarray(gate_b, np.float32))

    # Per-expert token lists + coefficients.
    toks, coefs = [], []
    for e in range(N_EXPERTS):
        r0 = np.nonzero(idx2[:, 0] == e)[0]
        r1 = np.nonzero(idx2[:, 1] == e)[0]
        toks.append(np.concatenate([r0, r1]))
        coefs.append(np.concatenate([p2[r0, 0], p2[r1, 1]]).astype(np.float32))

    max_load = max(len(t) for t in toks)
    C = max(512, -(-max_load // 256) * 256)  # pad to multiple of 256, >= 512

    if C not in _NC_CACHE:
        _NC_CACHE[C] = build_nc(C)
    nc = _NC_CACHE[C]

    w1 = np.asarray(w1, np.float32)
    w2 = np.asarray(w2, np.float32)
    w3 = np.asarray(w3, np.float32)
    b1 = np.asarray(b1, np.float32)
    b2 = np.asarray(b2, np.float32)
    b3 = np.asarray(b3, np.float32)

    in_maps = []
    for e in range(N_EXPERTS):
        n_e = len(toks[e])
        xg = np.zeros((C, D_MODEL), np.float32)
        xg[:n_e] = x_flat[toks[e]]
        xT = np.ascontiguousarray(xg.T).reshape(KD, P, C).astype(BF16)
        # w1s[m, p, k*P+j] = w1[k*P+p, m*P+j]
        w1s = np.ascontiguousarray(
            w1[e].reshape(KD, P, MH, P).transpose(2, 1, 0, 3).reshape(MH, P, KD * P)
        ).astype(BF16)
        w2s = np.ascontiguousarray(
            w2[e].reshape(KD, P, MH, P).transpose(2, 1, 0, 3).reshape(MH, P, KD * P)
        ).astype(BF16)
        # w3s[dm, p, kh*P+j] = w3[kh*P+p, dm*P+j]
        w3s = np.ascontiguousarray(
            w3[e].reshape(MH, P, MD, P).transpose(2, 1, 0, 3).reshape(MD, P, MH * P)
        ).astype(BF16)
        b1s = np.ascontiguousarray(b1[e].reshape(MH, P).T)
        b2s = np.ascontiguousarray(b2[e].reshape(MH, P).T)
        in_maps.append(
            {"xT": xT, "w1s": w1s, "w2s": w2s, "w3s": w3s, "b1s": b1s, "b2s": b2s}
        )

    res = run_bass_kernel_spmd(nc, in_maps, core_ids=list(range(N_EXPERTS)), trace=TRACE)
    LAST_RESULT = res

    out_flat = np.zeros((N, D_MODEL), np.float32)
    for e in range(N_EXPERTS):
        n_e = len(toks[e])
        yT = np.asarray(res.results[e]["yT"], np.float32).reshape(D_MODEL, C)
        y = yT.T[:n_e]  # [n_e, D]
        out_flat[toks[e]] += coefs[e][:, None] * (y + b3[e][None, :])

    return out_flat.reshape(x.shape), aux_loss
